# revision 4
# baseline (speedup 1.0000x reference)
"""Trainium2 Bass kernel v2 for the combined point-cloud loss.

Changes vs v1:
  - sqrt/exp/conf tails moved to host (kills ACT table-set flapping and the
    per-iteration tail ops); device emits raw squared-distance row mins.
  - Optional shift of some PSUM->SBUF drains from ACT (relu) to DVE
    (tensor_scalar_max) to balance the two engines (dve_shift groups of 16).
(tensor_tensor_reduce was tried and crashes the DVE exec unit on this
runtime -- NRT_EXEC_UNIT_UNRECOVERABLE -- so the row min stays a fold tree.)
"""

import numpy as np

import concourse.bacc as bacc
import concourse.tile as tile
from concourse import mybir
from concourse.bass_utils import run_bass_kernel_spmd

F16 = mybir.dt.float16
F32 = mybir.dt.float32
MIN = mybir.AluOpType.min
MAX = mybir.AluOpType.max
AX = mybir.AxisListType.X
AF = mybir.ActivationFunctionType

B = 4
N_UP = 8192
N_GT = 8192
N_RAD = 1024
HALF_UP = N_UP // 2
HALF_RAD = N_RAD // 2
UP_TILES = HALF_UP // 128    # 32
RAD_TILES = HALF_RAD // 128  # 4
GT_GROUPS = N_GT // 2048     # 4
N_CORES = 8

_NC_CACHE = {}


def _build_nc(loop_n=1, dve_shift=0, ablate="full"):
    from contextlib import ExitStack

    nc = bacc.Bacc("TRN2")
    up_p = nc.declare_dram_parameter("up_lhsT", [77, HALF_UP], F16, isOutput=False)
    rad_p = nc.declare_dram_parameter("rad_lhsT", [77, HALF_RAD], F16, isOutput=False)
    gt_p = nc.declare_dram_parameter("gt_rhs", [77, N_GT], F16, isOutput=False)
    ident_p = nc.declare_dram_parameter("ident", [128, 128], F16, isOutput=False)
    d2_p = nc.declare_dram_parameter("d2_out", [128, N_GT // 128], F32, isOutput=True)
    minsq_p = nc.declare_dram_parameter("minsq_out", [128, UP_TILES], F32, isOutput=True)
    minrad_p = nc.declare_dram_parameter(
        "minrad_out", [128, RAD_TILES], F32, isOutput=True
    )

    with ExitStack() as ctx:
        tc = ctx.enter_context(tile.TileContext(nc))
        singles = ctx.enter_context(tc.tile_pool(name="singles", bufs=1))
        psum = ctx.enter_context(tc.tile_pool(name="psum", bufs=2, space="PSUM"))
        stage = ctx.enter_context(tc.tile_pool(name="stage", bufs=3))

        up_sb = singles.tile([77, HALF_UP], F16)
        rad_sb = singles.tile([77, HALF_RAD], F16)
        gt_sb = singles.tile([77, N_GT], F16)
        ident_sb = singles.tile([128, 128], F16)
        nc.sync.dma_start(out=up_sb, in_=up_p[:])
        nc.sync.dma_start(out=rad_sb, in_=rad_p[:])
        nc.sync.dma_start(out=gt_sb, in_=gt_p[:])
        nc.sync.dma_start(out=ident_sb, in_=ident_p[:])

        colacc = singles.tile([128, N_GT], F16)
        minsq = singles.tile([128, UP_TILES], F32)
        minrad = singles.tile([128, RAD_TILES], F32)
        folds = ctx.enter_context(tc.tile_pool(name="folds", bufs=2))
        if ablate != "full":
            nc.vector.memset(minsq, 0.0)
            nc.vector.memset(minrad, 0.0)
            nc.vector.memset(colacc, 0.0)

        if loop_n > 1:
            ctx.enter_context(tc.For_i(0, loop_n, 1))

        group_counter = [0]

        def dist_tile(lhsT, accum_col, first, minsq_ap, rg=0):
            if ablate == "mmonly":
                st = None
            else:
                st = colacc if first else stage.tile([128, N_GT], F16, tag="st")
            for jg in range(GT_GROUPS):
                ps = psum.tile([128, 2048], F32, tag="ps")
                for jj in range(4):
                    c0 = jg * 2048 + jj * 512
                    nc.tensor.matmul(
                        ps[:, jj * 512 : (jj + 1) * 512],
                        lhsT=lhsT,
                        rhs=gt_sb[rg : rg + 13, c0 : c0 + 512],
                        start=True,
                        stop=True,
                    )
                if ablate == "mmonly":
                    continue
                dst = st[:, jg * 2048 : (jg + 1) * 2048]
                k = group_counter[0]
                group_counter[0] += 1
                if k % 16 < dve_shift:
                    nc.vector.tensor_scalar_max(dst, ps, 0.0)
                else:
                    nc.scalar.activation(out=dst, in_=ps, func=AF.Relu)
            if ablate != "full":
                return
            if accum_col and not first:
                nc.vector.tensor_tensor(colacc, colacc, st, MIN)
            # log2 folds along free dim, then a final 512-wide reduce
            f1 = folds.tile([128, 4096], F16, tag="f1")
            nc.vector.tensor_tensor(f1, st[:, :4096], st[:, 4096:], MIN)
            f2 = folds.tile([128, 2048], F16, tag="f2")
            nc.vector.tensor_tensor(f2, f1[:, :2048], f1[:, 2048:], MIN)
            f3 = folds.tile([128, 1024], F16, tag="f3")
            nc.vector.tensor_tensor(f3, f2[:, :1024], f2[:, 1024:], MIN)
            f4 = folds.tile([128, 512], F16, tag="f4")
            nc.vector.tensor_tensor(f4, f3[:, :512], f3[:, 512:], MIN)
            nc.vector.tensor_reduce(minsq_ap, f4, axis=AX, op=MIN)

        sched = []
        for i in range(UP_TILES):
            sched.append(("up", i))
            if i % 8 == 7:
                sched.append(("rad", i // 8))
        for k, (kind, i) in enumerate(sched):
            rg = (k % 3) * 32
            if kind == "up":
                dist_tile(
                    up_sb[rg : rg + 13, i * 128 : (i + 1) * 128],
                    accum_col=True,
                    first=(i == 0),
                    minsq_ap=minsq[:, i : i + 1],
                    rg=rg,
                )
            else:
                dist_tile(
                    rad_sb[rg : rg + 13, i * 128 : (i + 1) * 128],
                    accum_col=False,
                    first=False,
                    minsq_ap=minrad[:, i : i + 1],
                    rg=rg,
                )

        # dist2 partition-axis min: PE-transpose 128x128 blocks of colacc into
        # PSUM (gt on partitions), then free-axis reduce_min 8 blocks at a time.
        d2t = singles.tile([128, N_GT // 128], F32)
        if ablate != "full":
            nc.vector.memset(d2t, 0.0)
        for tq in range(N_GT // 1024 if ablate == "full" else 0):
            tp = psum.tile([128, 1024], F16, tag="ps")
            for tt in range(8):
                blk = tq * 8 + tt
                nc.tensor.transpose(
                    tp[:, tt * 128 : (tt + 1) * 128],
                    colacc[:, blk * 128 : (blk + 1) * 128],
                    ident_sb,
                )
            nc.vector.tensor_reduce(
                d2t[:, tq * 8 : (tq + 1) * 8],
                tp.rearrange("p (b f) -> p b f", f=128),
                axis=AX,
                op=MIN,
            )
        nc.sync.dma_start(out=d2_p[:], in_=d2t)
        nc.sync.dma_start(out=minsq_p[:], in_=minsq)
        nc.sync.dma_start(out=minrad_p[:], in_=minrad)

    nc.compile()
    return nc


def _get_nc():
    if "nc" not in _NC_CACHE:
        _NC_CACHE["nc"] = _build_nc()
    return _NC_CACHE["nc"]


def _split16(x):
    h = x.astype(np.float16)
    l = (x.astype(np.float64) - h.astype(np.float64)).astype(np.float16)
    return h, l


def _build_A(pts):
    # pts [N,3] fp32 -> lhsT [13, N] fp16
    n = pts.shape[0]
    ah, al = _split16(pts)
    a2 = np.sum(pts.astype(np.float64) ** 2, axis=1)
    a2h, a2l = _split16(a2)
    out = np.empty((13, n), dtype=np.float16)
    out[0:3] = ah.T
    out[3:6] = al.T
    out[6:9] = ah.T
    out[9] = a2h
    out[10] = a2l
    out[11] = 1.0
    out[12] = 1.0
    return out


def _build_B(pts):
    # pts [M,3] fp32 -> rhs [13, M] fp16
    m = pts.shape[0]
    bh, bl = _split16(pts)
    b2 = np.sum(pts.astype(np.float64) ** 2, axis=1)
    b2h, b2l = _split16(b2)
    out = np.empty((13, m), dtype=np.float16)
    out[0:3] = -2.0 * bh.T
    out[3:6] = -2.0 * bh.T
    out[6:9] = -2.0 * bl.T
    out[9] = 1.0
    out[10] = 1.0
    out[11] = b2h
    out[12] = b2l
    return out


def _dup77(m13):
    out = np.zeros((77, m13.shape[1]), dtype=np.float16)
    out[0:13] = m13
    out[32:45] = m13
    out[64:77] = m13
    return out


def _make_in_maps(pc_up, pc_conf, pc2, pc3):
    ident = np.eye(128, dtype=np.float16)
    in_maps = []
    for core in range(N_CORES):
        b, h = divmod(core, 2)
        up = pc_up[b, h * HALF_UP : (h + 1) * HALF_UP]
        rad = pc3[b, h * HALF_RAD : (h + 1) * HALF_RAD]
        in_maps.append(
            {
                "up_lhsT": _dup77(_build_A(up)),
                "rad_lhsT": _dup77(_build_A(rad)),
                "gt_rhs": _dup77(_build_B(pc2[b])),
                "ident": ident,
            }
        )
    return in_maps


def kernel(pc_up, pc_seed, pc_conf, pc2, pc3):
    del pc_seed  # unused by the reference loss
    nc = _get_nc()
    in_maps = _make_in_maps(pc_up, pc_conf, pc2, pc3)
    results = run_bass_kernel_spmd(nc, in_maps, list(range(N_CORES))).results

    # Host-side combine (the cheap "all-reduce" of the data-parallel sharding).
    tot_d1 = 0.0
    tot_sqrt = 0.0
    tot_d2 = 0.0
    tot_sse = 0.0
    for b in range(B):
        r0 = results[2 * b]
        r1 = results[2 * b + 1]
        # d2_out[p, t] corresponds to gt index t*128 + p
        d2 = np.minimum(
            r0["d2_out"].astype(np.float64), r1["d2_out"].astype(np.float64)
        )
        tot_d2 += d2.sum()
        for h, r in ((0, r0), (1, r1)):
            ms = np.maximum(r["minsq_out"].astype(np.float64), 0.0)
            tot_d1 += ms.sum()
            tot_sqrt += np.sqrt(ms).sum()
            mr = np.maximum(r["minrad_out"].astype(np.float64), 0.0)
            # minrad_out[p, t] is radar point index h*HALF_RAD + t*128 + p
            scores = np.exp(-np.sqrt(mr))
            conf = (
                pc_conf[b, h * HALF_RAD : (h + 1) * HALF_RAD, 0]
                .astype(np.float64)
                .reshape(RAD_TILES, 128)
                .T
            )
            tot_sse += ((conf - scores) ** 2).sum()

    m1 = tot_d1 / (B * N_UP)
    m2 = tot_d2 / (B * N_GT)
    emd = tot_sqrt / (B * N_UP)
    conf_mse = tot_sse / (B * N_RAD)
    alpha = 0.5
    chamfer = 0.5 * m1 + 2.0 * m2
    final = alpha * chamfer + alpha * conf_mse + emd
    return np.array(final, dtype=np.float32)
